# revision 77
# baseline (speedup 1.0000x reference)
"""Causal self-attention Bass/Tile kernel for Trainium2, 8 NeuronCores.

Problem: B=4, T=2048, C=1024, NH=16, HD=64.
  q/k/v = x @ W{q,k,v}; att = softmax(causal(q k^T / 8)); y = (att v) @ Wp

Sharding (8 cores): batch (4-way) x head-group (2-way tensor parallel).
Core c handles batch b=c//2 and global heads g*8..g*8+7 where g=c%2.
Each core computes a partial projection y_part = y_heads_local @ Wp[rows]
and the host unshards by summing the two partial outputs per batch.

Per-core kernel (all T=2048 tokens, 8 heads, head_dim 64), bf16 matmuls
with fp32 PSUM accumulation:
  The emit order software-pipelines everything around the two pacing
  engines: PE (matmul, the roofline engine at ~204us busy) and ACT
  (exp, ~149us).  Scores are computed per query tile j (512 wide) /
  head pair pr as transposed tiles S^T [s:128, t], with the causal
  region tightened at 128 granularity (diagonal s-tiles only compute
  the suffix t-window).  exp(S/8) runs on ACT (PSUM->SBUF bf16); the
  128x128 true-diagonal blocks are masked post-exp by small GPSIMD
  affine_selects, pipelined behind the exp stream.

  PV accumulates t-major: out y[t128, hi, 0:65] per head with lhsT =
  the exp tile (stationary) and rhs = [V | 1] (moving, 65 wide), so
  each matmul costs 65 PE rows instead of 512 (TimelineSim charges
  N = moving free-size only).  The ones column lands the softmax
  denominator in column 64; normalize is then a fp32 reciprocal along
  the free dim (no cross-partition moves) + one stride-0-broadcast
  DVE multiply per half.  PE transposes (identity built on-chip)
  rebuild ylocT [u, t] for the output projection y = ylocT^T @ Wp.
  Only the first matmul touching each PSUM bank carries
  start_tensor_calc (it marks the whole 2KB zero region pending-zero).

  The QK score stream is throttled to the exp pace by the 2-slot PSUM
  rotation, so each slot also runs the PV matmuls for the s-tile four
  slots back plus debt-carried filler (qT/kT/v projection tiles keyed
  to the block that consumes them, output-projection tiles reserved
  for the long j=3 blocks, deferred transpose tails).  DMAs serialize
  on one modeled pipe in emission order: first-needed weight pairs and
  x chunks lead (q/k weights host-packed into their exact SBUF layout
  to get >=512B descriptor runs), wp trails.  y is staged bf16 and the
  final tiles split across two queues/copy engines to shrink the
  drain; the host sums the two TP partials in fp32.
"""

import numpy as np

B, T, C, NH, HD = 4, 2048, 1024, 16, 64
G = 512          # local head dims per core (8 heads x 64)
P = 128
NT = 4           # t tiles of 512
NT128 = 16       # t tiles of 128
NPAIR = 4        # local head pairs
TT = 512

_CACHE = {}


def _build_nc():
    import concourse.tile as tile
    from concourse import bacc, mybir
    from concourse.bass import AP as BassAP

    f32 = mybir.dt.float32
    bf16 = mybir.dt.bfloat16

    nc = bacc.Bacc("TRN2", target_bir_lowering=False, debug=False)

    xT = nc.dram_tensor("xt", [C, T], bf16, kind="ExternalInput")
    # host-packed q/k weights in the exact SBUF layout [p, slot, co, 128]
    # (slot 2*dg+view) so each per-pair DMA is one >=512B-run transfer
    wqk = nc.dram_tensor("wqk", [P, 8, 8, P], bf16, kind="ExternalInput")
    wv = nc.dram_tensor("wv", [C, G], bf16, kind="ExternalInput")
    wp = nc.dram_tensor("wp", [G, C], bf16, kind="ExternalInput")
    y = nc.dram_tensor("y", [T, C], bf16, kind="ExternalOutput")

    xT_v = xT.rearrange("(co p) t -> p co t", p=P)      # [128, 8, 2048]
    wv_v = wv.rearrange("(co p) g -> p co g", p=P)      # [128, 8, 512]
    wp_v = wp.rearrange("(uo p) c -> p uo c", p=P)      # [128, 4, 1024]
    y_v = y.rearrange("(to p) c -> p to c", p=P)        # [128, 16, 1024]

    with tile.TileContext(nc) as tc:
        with (
            tc.tile_pool(name="singles", bufs=1) as singles,
            tc.tile_pool(name="expst", bufs=3) as epool,
            tc.tile_pool(name="norm", bufs=4) as npool,
            tc.tile_pool(name="rcps", bufs=2) as rcpool,
            tc.tile_pool(name="ystage", bufs=3) as ypool,
            tc.tile_pool(name="psS", bufs=2, space="PSUM") as psS,
            tc.tile_pool(name="psV", bufs=2, space="PSUM") as psV,
            tc.tile_pool(name="psA", bufs=2, space="PSUM") as psA,
        ):
            # persistent tensors
            xT_sb = singles.tile([P, 8, T], bf16, name="xT_sb", tag="xT_sb")
            # wqk_sb[:, 2*dg+view, co, :]: lhsT tiles for q (view 0), k (view 1)
            wqk_sb = singles.tile([P, 8, 8, P], bf16, name="wqk_sb", tag="wqk_sb")
            wv_sb = singles.tile([P, 8, G], bf16, name="wv_sb", tag="wv_sb")
            wp_sb = singles.tile([P, NPAIR, C], bf16, name="wp_sb", tag="wp_sb")
            qT = singles.tile([P, NPAIR, T], bf16, name="qT", tag="qT")
            kT = singles.tile([P, NPAIR, T], bf16, name="kT", tag="kT")
            v_sb = singles.tile([P, NT128, 8, 66], bf16, name="v_sb", tag="v_sb")
            ylocT = singles.tile([P, NPAIR, T], bf16, name="ylocT", tag="ylocT")
            # identity (for PE transposes), built by masking an all-ones tile
            ident = singles.tile([P, P], bf16, name="ident", tag="ident")

            nc.vector.memset(ident[:], 1.0)
            nc.vector.memset(v_sb[:, :, :, 64:65], 1.0)
            # All DMAs serialize on one modeled DMA pipe in gen-completion
            # order, so the emission order here IS the arrival priority:
            # wqk pair0 + xT jj0 first (PE start), remaining pairs, wv
            # (needed by the prologue v tiles ~20us in), then the rest of
            # xT, and wp (first needed >60us in) last on the ACT queue.
            nc.sync.dma_start(wqk_sb[:, 0:2, 0:4, :], wqk[:, 0:2, 0:4, :])
            nc.scalar.dma_start(
                xT_sb[:, 0:2, 0:TT], xT_v[:, 0:2, 0:TT])
            nc.sync.dma_start(wqk_sb[:, 0:2, 4:8, :], wqk[:, 0:2, 4:8, :])
            nc.scalar.dma_start(
                xT_sb[:, 2:4, 0:TT], xT_v[:, 2:4, 0:TT])
            nc.sync.dma_start(xT_sb[:, 4:8, 0:TT], xT_v[:, 4:8, 0:TT])
            for dg in range(1, NPAIR):
                nc.sync.dma_start(
                    wqk_sb[:, 2 * dg:2 * dg + 2, :, :],
                    wqk[:, 2 * dg:2 * dg + 2, :, :])
                nc.scalar.dma_start(
                    xT_sb[:, 4:8, dg * TT:(dg + 1) * TT],
                    xT_v[:, 4:8, dg * TT:(dg + 1) * TT])
            for ch in range(2):
                nc.sync.dma_start(
                    wv_sb[:, 4 * ch:4 * ch + 4, :], wv_v[:, 4 * ch:4 * ch + 4, :])
            for jj in range(1, NT):
                nc.sync.dma_start(
                    xT_sb[:, 0:4, jj * TT:(jj + 1) * TT],
                    xT_v[:, 0:4, jj * TT:(jj + 1) * TT])
            for ch in range(2):
                nc.sync.dma_start(
                    wp_sb[:, 2 * ch:2 * ch + 2, :], wp_v[:, 2 * ch:2 * ch + 2, :])

            # p-state warmup: the cost model runs matmuls at 0.65/1.2GHz
            # until the PE has been continuously busy for 3us, so burn tiny
            # identity matmuls from t~0.7us until the first weights land
            # (~4.4us); the real stream then starts at full 2.4GHz.
            ps_w = psA.tile([P, P], f32, name="ps_w", tag="psA")
            for _ in range(70):
                nc.tensor.matmul(ps_w[:], ident[:], ident[:],
                                 start=True, stop=True)
            # carve the all-ones warmup tile into the transpose identity
            # only after the dummies have read it (first use is ~30us in)
            nc.gpsimd.affine_select(
                out=ident[:], in_=ident[:], pattern=[[1, P]],
                compare_op=mybir.AluOpType.is_ge, fill=0.0,
                base=0, channel_multiplier=-1)
            nc.gpsimd.affine_select(
                out=ident[:], in_=ident[:], pattern=[[-1, P]],
                compare_op=mybir.AluOpType.is_ge, fill=0.0,
                base=0, channel_multiplier=1)

            # ---------- emit helpers for PE work units ----------
            def emit_qk_tile(view, dg, jj):
                dstT = qT if view == 0 else kT
                ps = psA.tile([P, TT], f32, name="ps_qk", tag="psA")
                for co in range(8):
                    nc.tensor.matmul(
                        ps[:], wqk_sb[:, 2 * dg + view, co, :],
                        xT_sb[:, co, jj * TT:(jj + 1) * TT],
                        start=(co == 0), stop=(co == 7))
                nc.vector.tensor_copy(
                    out=dstT[:, dg, jj * TT:(jj + 1) * TT], in_=ps[:])

            def emit_v_tile(t128):
                ps = psA.tile([P, G], f32, name="ps_v", tag="psA")
                for co in range(8):
                    nc.tensor.matmul(
                        ps[:], xT_sb[:, co, t128 * P:(t128 + 1) * P],
                        wv_sb[:, co, :],
                        start=(co == 0), stop=(co == 7))
                nc.vector.tensor_copy(
                    out=v_sb[:, t128, :, 0:64],
                    in_=ps.rearrange("p (h d) -> p h d", h=8))

            def emit_d_tile(t128, cn, pool=None):
                ps = (pool or psA).tile([P, TT], f32, name="ps_y",
                                        tag="psV" if pool is psV else "psA")
                for uo in range(4):
                    nc.tensor.matmul(
                        ps[:],
                        ylocT[:, uo, t128 * P:(t128 + 1) * P],
                        wp_sb[:, uo, cn * TT:(cn + 1) * TT],
                        start=(uo == 0), stop=(uo == 3))
                yst = ypool.tile([P, TT], bf16, name="yst", tag="yst")
                # tail tiles split across two engines/queues to shrink the
                # final drain (the ACT engine+queue are free of exps by then)
                tail = t128 >= 12 and cn == 1
                if tail:
                    nc.scalar.copy(out=yst[:], in_=ps[:])
                else:
                    nc.vector.tensor_copy(out=yst[:], in_=ps[:])
                q = nc.scalar if tail else nc.sync
                q.dma_start(
                    out=y_v[:, t128, cn * TT:(cn + 1) * TT], in_=yst[:])

            # normalized-yloc transpose: 4 PE transposes rebuild the
            # [u, t] orientation the output projection needs
            def emit_tr(j, pr, yvs):
                psT = psA.tile([P, 4, P], bf16, name="psT", tag="psA")
                for half, yv in enumerate(yvs):
                    for tqs in range(2):
                        nc.tensor.transpose(
                            psT[:, 2 * half + tqs, :],
                            yv[:, tqs, :, :], ident[:])
                nc.vector.tensor_copy(
                    out=ylocT[:, pr, j * TT:(j + 1) * TT],
                    in_=psT[:])
                if pr == 3:
                    for t in range(4 * j, 4 * j + 4):
                        for cn in range(2):
                            fillerD.append(
                                (860, (lambda tt=t, c=cn: emit_d_tile(tt, c))))

            # ---------- filler scheduling ----------
            # fillerA: remaining projection tiles ordered per consuming
            # block: v tiles for j's PV first (forced at B(j, 0) start),
            # then (k, q) pairs per head pair (forced at B(j, pr) start).
            # Items left over feed the debt-carried pulls; fillerD (output
            # tiles) is reserved for the j=3 blocks, whose exp stream is
            # the longest and would otherwise leave PE idle.
            fillerA = []     # (key, cost_ns, fn); key = (jj, kind, dg)
            for jj in range(1, NT):
                for dg in range(NPAIR):
                    fillerA.append(
                        ((jj, 1, dg), 1710,
                         (lambda d=dg, t=jj: emit_qk_tile(1, d, t))))
                    fillerA.append(
                        ((jj, 2, dg), 1710,
                         (lambda d=dg, t=jj: emit_qk_tile(0, d, t))))
                for tq in range(4):
                    fillerA.append(
                        ((jj, 3, tq), 1710,
                         (lambda t=4 * jj + tq: emit_v_tile(t))))
            fillerD = []     # (cost_ns, fn)
            c2q = []         # pending transpose tails
            debt = [0.0]

            d_reserve = [0]

            def pull(ns, d_ok=False):
                debt[0] += ns
                while debt[0] > 0:
                    if fillerA:
                        _, cost, fn = fillerA.pop(0)
                    elif fillerD and d_ok and len(fillerD) > d_reserve[0]:
                        cost, fn = fillerD.pop(0)
                    else:
                        debt[0] = 0.0
                        return
                    fn()
                    debt[0] -= cost

            def drain_c2():
                while c2q:
                    c2q.pop(0)()

            def drain_A(upto_key):
                while fillerA and fillerA[0][0] <= upto_key:
                    _, _, fn = fillerA.pop(0)
                    fn()

            # ---------- attention block for one (j, pr) ----------
            def emit_B(j, pr):
                # keep a few output tiles in reserve for the final block,
                # whose own stream otherwise runs dry
                d_reserve[0] = 3 if (j == 3 and pr < 3) else 0
                if j >= 1:
                    drain_A((j, 2, pr))
                ns = 4 * (j + 1)
                so_list = list(range(4 * j)) + list(range(4 * j, 4 * j + 4))
                expp_lo = epool.tile(
                    [P, 8, 2, TT], bf16, name="expp_lo", tag="expp")
                expp_hi = expp_lo if ns <= 8 else epool.tile(
                    [P, 8, 2, TT], bf16, name="expp_hi", tag="expp")

                def etile(so):
                    return expp_lo if so < 8 else expp_hi

                # Fused QK/exp/PV stream (diagonal s-tiles last,
                # tightened windows).  The 2-slot psS rotation throttles QK
                # to the exp pace; each slot also runs the PV matmuls for
                # the s-tile FOUR slots back (exp landed, and the previous
                # block's normalize has had time to free the PV PSUM
                # slots), plus debt-carried filler for the deficit.
                # PV accumulates t-major: out y[t128, hi, 65] with the
                # softmax denominator in column 64 (the [V | 1] ones
                # column), two t128 chunks per single-bank PSUM pass.
                # start_tensor_calc marks the WHOLE 2KB PSUM zero region
                # pending-zero, so only the first matmul touching each bank
                # carries it; every region's own first write is then
                # zero-filled (not accumulated) automatically.
                ps_vs = [psV.tile([P, 2, 2, 65], f32, name="ps_v", tag="psV")
                         for _ in range(2)]
                first_mm = [True, True]

                def emit_pv(so):
                    a = so - 4 * j
                    # descending tq: the mask-dependent diagonal block
                    # (tq == a) goes last, giving its mask extra slack
                    for tq in range(3, -1, -1):
                        if a >= 0 and tq < a:
                            continue
                        half, tqs = tq // 2, tq % 2
                        for hi in range(2):
                            nc.tensor.matmul(
                                ps_vs[half][:, tqs, hi, 0:65],
                                etile(so)[:, so % 8, hi,
                                          tq * P:(tq + 1) * P],
                                v_sb[:, so, 2 * pr + hi, 0:65],
                                start=first_mm[half], stop=(a == tq),
                                skip_group_check=True)
                            first_mm[half] = False

                for si, so in enumerate(so_list):
                    a = so - 4 * j
                    off = 128 * a if a >= 0 else 0
                    ps_s = psS.tile([P, 2, TT], f32, name="ps_s", tag="psS")
                    for hi in range(2):
                        hp = 64 * hi
                        nc.tensor.matmul(
                            ps_s[:, hi, off:TT],
                            kT[hp:hp + 64, pr, so * P:(so + 1) * P],
                            qT[hp:hp + 64, pr, j * TT + off:(j + 1) * TT],
                            start=True, stop=True)
                    nc.scalar.activation(
                        out=etile(so)[:, so % 8, :, off:TT],
                        in_=ps_s[:, :, off:TT],
                        func=mybir.ActivationFunctionType.Exp,
                        scale=0.125)
                    if a >= 0:
                        # mask the 128x128 true-diagonal blocks of both heads
                        # in one op (s > t -> 0; the hi dim gets multiplier 0)
                        blk = etile(so)[:, so % 8, :, off:off + P]
                        nc.gpsimd.affine_select(
                            out=blk, in_=blk,
                            pattern=[[0, 2], [1, P]],
                            compare_op=mybir.AluOpType.is_ge,
                            fill=0.0, base=0, channel_multiplier=-1)
                    if si == 2:
                        drain_c2()
                    if si == ns - 2:
                        drain_A((j, 3, 0))
                    if si >= 4:
                        emit_pv(so_list[si - 4])
                    if si >= 1:
                        pull(380, d_ok=(j == 3))
                # normalize per half as soon as its last PV lands (half A
                # is complete after diagonal a=1): fp32 reciprocal of the
                # denominators (free column 64 -- no partition crossing),
                # then one broadcast multiply into bf16 (the reciprocal
                # column is stride-0-expanded across the 64 dims).  Early
                # half-A normalize recycles its PSUM slot two slots sooner.
                yvs = []

                def emit_norm(half):
                    rcp = rcpool.tile([P, 2, 2, 1], f32, name="rcp",
                                      tag=f"rcp{half}")
                    nc.vector.reciprocal(
                        out=rcp[:], in_=ps_vs[half][:, :, :, 64:65])
                    rsl = rcp[:, :, :, 0:1]
                    rb = BassAP(rsl.tensor, rsl.offset,
                                [list(rsl.ap[0]), [2, 2], [1, 2], [0, 64]])
                    yv = npool.tile([P, 2, 2, 64], bf16, name="yv", tag="yv")
                    nc.vector.tensor_tensor(
                        out=yv[:], in0=ps_vs[half][:, :, :, 0:64], in1=rb,
                        op=mybir.AluOpType.mult)
                    yvs.append(yv)

                drain_A((j, 3, 3))
                for k in range(max(0, ns - 4), ns):
                    emit_pv(so_list[k])
                    if k == ns - 3:
                        emit_norm(0)
                    pull(220 if j < 3 else 280, d_ok=(j == 3))
                emit_norm(1)
                if j == NT - 1 and pr == NPAIR - 1:
                    # final block: per-half transpose + copy + output tiles,
                    # so the first half's output projection overlaps the
                    # second half's normalize/transpose chain
                    for half in range(2):
                        # psV slots are free after this half's normalize;
                        # keep psA exclusively for the output tiles
                        psT = psV.tile([P, 2, P], bf16, name="psTh", tag="psV")
                        for tqs in range(2):
                            nc.tensor.transpose(
                                psT[:, tqs, :],
                                yvs[half][:, tqs, :, :], ident[:])
                        nc.vector.tensor_copy(
                            out=ylocT[:, pr, (2 * j + half) * 2 * P:
                                      (2 * j + half + 1) * 2 * P],
                            in_=psT[:])
                        for ti, t in enumerate(
                                (4 * j + 2 * half, 4 * j + 2 * half + 1)):
                            for cn in range(2):
                                # alternate PSUM pools (psV is free after the
                                # last normalize) so the 4-tile run is not
                                # throttled by the 2-slot psA rotation
                                emit_d_tile(t, cn)
                else:
                    c2q.append(
                        lambda jj=j, pp=pr, ys=yvs: emit_tr(jj, pp, ys))

            # ---------- main emit ----------
            # prologue: everything B(0) needs
            for dg in range(NPAIR):
                emit_qk_tile(0, dg, 0)
                emit_qk_tile(1, dg, 0)
            for tq in range(4):
                emit_v_tile(tq)
            for j in range(NT):
                for pr in range(NPAIR):
                    emit_B(j, pr)
            # tail: remaining normalize chains, projections, output tiles
            drain_c2()
            drain_A((NT, 3, NPAIR))
            while fillerD:
                _, fn = fillerD.pop(0)
                fn()

    nc.finalize()
    return nc


def _get_nc():
    if "nc" not in _CACHE:
        _CACHE["nc"] = _build_nc()
    return _CACHE["nc"]


def _pack_wqk(Wq_sl, Wk_sl):
    """[C, G] q/k weight slices -> [128, 8, 8, 128]: [p, 2*dg+view, co, g]."""
    wqs = Wq_sl.reshape(8, P, NPAIR, P).transpose(1, 2, 0, 3)  # [p, dg, co, g]
    wks = Wk_sl.reshape(8, P, NPAIR, P).transpose(1, 2, 0, 3)
    packed = np.empty((P, 8, 8, P), np.float32)
    packed[:, 0::2] = wqs
    packed[:, 1::2] = wks
    return packed


def shard_inputs(x, Wq, Wk, Wv, Wp):
    """Build the 8 per-core input maps."""
    import ml_dtypes
    bf = ml_dtypes.bfloat16
    x = np.asarray(x, np.float32)
    Wq, Wk, Wv, Wp = (np.asarray(w, np.float32) for w in (Wq, Wk, Wv, Wp))
    in_maps = []
    for c in range(8):
        b, g = c // 2, c % 2
        sl = slice(g * G, (g + 1) * G)
        in_maps.append({
            "xt": np.ascontiguousarray(x[b].T).astype(bf),
            "wqk": _pack_wqk(Wq[:, sl], Wk[:, sl]).astype(bf),
            "wv": np.ascontiguousarray(Wv[:, sl]).astype(bf),
            "wp": np.ascontiguousarray(Wp[sl, :]).astype(bf),
        })
    return in_maps


def unshard_outputs(results):
    """results: list of 8 dicts with 'y' [T, C] bf16 partials -> [B, T, C]."""
    out = np.empty((B, T, C), np.float32)
    for b in range(B):
        out[b] = (np.asarray(results[2 * b]["y"], np.float32)
                  + np.asarray(results[2 * b + 1]["y"], np.float32))
    return out


def kernel(**inputs):
    from concourse import bass_utils
    nc = _get_nc()
    in_maps = shard_inputs(**inputs)
    res = bass_utils.run_bass_kernel_spmd(nc, in_maps, core_ids=list(range(8)))
    return unshard_outputs(res.results)
